# revision 3
# baseline (speedup 1.0000x reference)
"""Trainium2 Bass kernel for per-batch spatial self-attention (fp8/bf16).

Per-core computation (one batch image per NeuronCore, 8 cores):
  x:(256, 4096) -> q/k = W x + b (channels-major, cast to fp8e4)
  St[m,n] = sum_c K8[c,m] Q8[c,n]   via fp8 DoubleRow (256-deep contraction
                                    in one matmul, 2x f32r throughput)
  Pt = exp(St/16) in bf16 (1024-wide activations over 2-bank PSUM tiles)
  OT[o,n] = sum_m Vb[m,o] Pt[m,n]   bf16 matmuls, channels-major output
  rowsum via fp16 DVE accumulation (2x mode) + ones-matmul over partitions
  out = OT / rowsum

Accuracy: q/k quantized to fp8e4 after f32r projections; pt/v in bf16.
Simulated rel_err vs f32 reference ~1.55e-2 (gate 2e-2).
"""

import sys

sys.path.insert(0, "/opt/trn_rl_repo")

import numpy as np
import concourse.bacc as bacc
import concourse.mybir as mybir
import concourse.tile as tile
from concourse.bass_utils import run_bass_kernel_spmd

F32 = mybir.dt.float32
F32R = mybir.dt.float32r
BF16 = mybir.dt.bfloat16
FP16 = mybir.dt.float16
F8 = mybir.dt.float8e4
AF = mybir.ActivationFunctionType
PM = mybir.MatmulPerfMode

B = 8
C = 256  # channels
NPIX = 4096  # 64*64
NT = NPIX // 512  # 8 column tiles of 512 queries
M = NPIX // 128  # 32 key chunks of 128
NJ = M // 2  # 16 key pair-tiles of 256
SCALE = 1.0 / 16.0  # 1/sqrt(C)

_CACHE = {}


def _build():
    nc = bacc.Bacc("TRN2", num_swdge_queues=4)
    x_d = nc.declare_dram_parameter("x", [C, NPIX], F32, isOutput=False)
    wq_d = nc.declare_dram_parameter("wq_t", [C, C], F32, isOutput=False)
    wk_d = nc.declare_dram_parameter("wk_t", [C, C], F32, isOutput=False)
    wv_d = nc.declare_dram_parameter("wv_t", [C, C], F32, isOutput=False)
    bq_d = nc.declare_dram_parameter("bq", [C, 1], F32, isOutput=False)
    bk_d = nc.declare_dram_parameter("bk", [C, 1], F32, isOutput=False)
    bv_d = nc.declare_dram_parameter("bv", [1, C], F32, isOutput=False)
    out_d = nc.declare_dram_parameter("out", [C, NPIX], F32, isOutput=True)

    with tile.TileContext(nc) as tc:
        with (
            tc.tile_pool(name="big", bufs=1) as big,
            tc.tile_pool(name="small", bufs=2) as small,
            tc.tile_pool(name="ptp", bufs=6) as ptp,
            tc.tile_pool(name="outp", bufs=4) as outp,
            tc.tile_pool(name="psA", bufs=2, space="PSUM") as psA,  # 2x[128,1024]=4 banks
            tc.tile_pool(name="psB", bufs=2, space="PSUM") as psB,  # ot0/ot1 = 2 banks
            tc.tile_pool(name="psR", bufs=1, space="PSUM") as psR,  # 1 bank
        ):
            import concourse.bass as bass

            # ---- load inputs (gpsimd DMA casts f32 -> f32r), need-ordered ----
            w_r = {}
            for nm, wd in (("q", wq_d), ("k", wk_d), ("v", wv_d)):
                w_r[nm] = [
                    big.tile([128, C], F32R, name=f"w{nm}_r{i}") for i in range(2)
                ]
            x_r = [big.tile([128, NPIX], F32R, name=f"x_r{i}") for i in range(2)]

            for i in range(2):
                nc.gpsimd.dma_start(
                    out=w_r["q"][i], in_=wq_d[i * 128 : (i + 1) * 128, :]
                )
            for i in range(2):
                nc.gpsimd.dma_start(
                    out=x_r[i][:, 0:512], in_=x_d[i * 128 : (i + 1) * 128, 0:512]
                )
            for i in range(2):
                nc.gpsimd.dma_start(
                    out=w_r["k"][i], in_=wk_d[i * 128 : (i + 1) * 128, :]
                )
            for i in range(2):
                nc.gpsimd.dma_start(
                    out=x_r[i][:, 512:1024],
                    in_=x_d[i * 128 : (i + 1) * 128, 512:1024],
                )
            for i in range(2):
                nc.gpsimd.dma_start(
                    out=w_r["v"][i], in_=wv_d[i * 128 : (i + 1) * 128, :]
                )
            for j in range(2, 8):
                lo, hi = j * 512, (j + 1) * 512
                for i in range(2):
                    nc.gpsimd.dma_start(
                        out=x_r[i][:, lo:hi], in_=x_d[i * 128 : (i + 1) * 128, lo:hi]
                    )
            bq_sb = [big.tile([128, 1], F32, name=f"bq_sb{i}") for i in range(2)]
            for i in range(2):
                nc.sync.dma_start(out=bq_sb[i], in_=bq_d[i * 128 : (i + 1) * 128, :])

            # bv broadcast to 128 partitions x 4 repeats (for 4-chunk V groups)
            bv_bc4 = big.tile([128, 4, C], F32, name="bv_bc4")
            for rep in range(4):
                bv_bcast_ap = bass.AP(
                    tensor=bv_d.ap().tensor,
                    offset=0,
                    ap=[[0, 128], [1, C]],
                )
                nc.sync.dma_start(out=bv_bc4[:, rep, :], in_=bv_bcast_ap)

            ones_f = big.tile([128, 1], F32, name="ones_f")
            nc.vector.memset(ones_f, 1.0)
            ones_h = big.tile([128, 1], FP16, name="ones_h")
            nc.vector.tensor_copy(ones_h, ones_f)
            ones_rf = big.tile([1, 128], F32, name="ones_rf")
            nc.vector.memset(ones_rf, 1.0)
            ones_row = big.tile([1, 128], F32R, name="ones_row")
            nc.vector.tensor_copy(ones_row, ones_rf)
            ones_col = big.tile([128, 1], F32R, name="ones_col")
            nc.vector.tensor_copy(ones_col, ones_f)

            # keep the PE busy (HAM clock-gate warm) while inputs stream in
            warm_f = small.tile([128, 256], F32, name="warm_f", tag="warm_f")
            nc.vector.memset(warm_f, 1.0)
            warm_r = small.tile([128, 256], F32R, name="warm_r", tag="warm_r")
            nc.vector.tensor_copy(warm_r, warm_f)
            warm_ps = psR.tile([1, 256], F32, name="warm_ps", tag="psR")
            for _ in range(48):
                nc.tensor.matmul(
                    warm_ps, ones_col, warm_r, start=True, stop=True,
                    skip_group_check=True,
                )

            # ---- Q, K projections -> fp8 pair-layout tiles [128, 2, NPIX] ----
            q8 = big.tile([128, 2, NPIX], F8, name="q8")
            k8 = big.tile([128, 2, NPIX], F8, name="k8")
            for nt in range(NT):
                for tgt, wkey, bias in ((q8, "q", bq_sb), (k8, "k", None)):
                    ps = psA.tile([128, 1024], F32, name="ps_proj", tag="psA")
                    for o in range(2):
                        for i in range(2):
                            nc.tensor.matmul(
                                ps[:, o * 512 : (o + 1) * 512],
                                w_r[wkey][i][:, o * 128 : (o + 1) * 128],
                                x_r[i][:, nt * 512 : (nt + 1) * 512],
                                start=(i == 0),
                                stop=(i == 1),
                                skip_group_check=True,
                            )
                    for o in range(2):
                        if bias is not None:
                            nc.scalar.activation(
                                tgt[:, o, nt * 512 : (nt + 1) * 512],
                                ps[:, o * 512 : (o + 1) * 512],
                                AF.Identity,
                                bias=bias[o],
                            )
                        else:
                            # k bias is softmax-invariant; skip it
                            nc.scalar.activation(
                                tgt[:, o, nt * 512 : (nt + 1) * 512],
                                ps[:, o * 512 : (o + 1) * 512],
                                AF.Copy,
                            )

            # ---- V projection (bf16, pixels-major), 4 chunks per psum tile ----
            vb = big.tile([128, M, C], BF16, name="vb")
            for g in range(M // 4):
                ps = psA.tile([128, 1024], F32, name="ps_v", tag="psA")
                for q in range(4):
                    m = 4 * g + q
                    for i in range(2):
                        nc.tensor.matmul(
                            ps[:, q * 256 : (q + 1) * 256],
                            x_r[i][:, m * 128 : (m + 1) * 128],
                            w_r["v"][i],
                            start=(i == 0),
                            stop=(i == 1),
                            skip_group_check=True,
                        )
                nc.vector.tensor_add(
                    vb[:, 4 * g : 4 * g + 4, :], ps, bv_bc4
                )

            # ---- attention, one 512-query tile at a time ----
            for nt in range(NT):
                ot = [
                    psB.tile([128, 512], F32, name=f"ot{o}", tag="psB")
                    for o in range(2)
                ]
                acc = small.tile([128, 1024], FP16, name="acc", tag="acc")
                LAG = 3
                ptws = {}
                for jj in range(NJ + LAG):
                    if jj < NJ:
                        j = jj
                        st = psA.tile([128, 1024], F32, name="st", tag="psA")
                        for i in range(2):
                            nc.tensor.matmul(
                                st[:, i * 512 : (i + 1) * 512],
                                k8[:, :, (2 * j + i) * 128 : (2 * j + i + 1) * 128],
                                q8[:, :, nt * 512 : (nt + 1) * 512],
                                start=True,
                                stop=True,
                                perf_mode=PM.DoubleRow,
                                skip_group_check=True,
                            )
                        ptw = ptp.tile([128, 1024], BF16, name="ptw")
                        nc.scalar.activation(ptw, st, AF.Exp, scale=SCALE)
                        ptws[j] = ptw
                    if jj >= LAG:
                        j = jj - LAG
                        ptw = ptws.pop(j)
                        if j == 0:
                            nc.vector.tensor_copy(acc, ptw)
                        else:
                            nc.vector.tensor_add(acc, acc, ptw)
                        for i in range(2):
                            m = 2 * j + i
                            for o in range(2):
                                nc.tensor.matmul(
                                    ot[o],
                                    vb[:, m, o * 128 : (o + 1) * 128],
                                    ptw[:, i * 512 : (i + 1) * 512],
                                    start=(m == 0),
                                    stop=(m == M - 1),
                                )
                # rowsum over partitions (both acc halves accumulate)
                rs = psR.tile([1, 512], F32, name="rs", tag="psR")
                for i in range(2):
                    nc.tensor.matmul(
                        rs,
                        ones_h,
                        acc[:, i * 512 : (i + 1) * 512],
                        start=(i == 0),
                        stop=(i == 1),
                    )
                rinv_f = small.tile([1, 512], F32, name="rinv_f", tag="rinv_f")
                nc.vector.reciprocal_approx_fast(rinv_f, rs)
                rinv = small.tile([1, 512], F32R, name="rinv", tag="rinv")
                nc.vector.tensor_copy(rinv, rinv_f)
                rb = psR.tile([128, 512], F32, name="rb", tag="psR")
                nc.tensor.matmul(rb, ones_row, rinv, start=True, stop=True)
                rb_sb = small.tile([128, 512], F32, name="rb_sb", tag="rb_sb")
                nc.vector.tensor_copy(rb_sb, rb)
                for o in range(2):
                    osb = outp.tile([128, 512], F32, name="osb", tag="osb")
                    nc.vector.tensor_mul(osb, ot[o], rb_sb)
                    nc.sync.dma_start(
                        out=out_d[o * 128 : (o + 1) * 128, nt * 512 : (nt + 1) * 512],
                        in_=osb,
                    )

    nc.compile()
    return nc


def _get_nc():
    if "nc" not in _CACHE:
        _CACHE["nc"] = _build()
    return _CACHE["nc"]


def kernel(x, wq, wk, wv, bq, bk, bv):
    x = np.asarray(x, dtype=np.float32)
    wq = np.asarray(wq, dtype=np.float32)
    wk = np.asarray(wk, dtype=np.float32)
    wv = np.asarray(wv, dtype=np.float32)
    bq = np.asarray(bq, dtype=np.float32)
    bk = np.asarray(bk, dtype=np.float32)
    bv = np.asarray(bv, dtype=np.float32)

    nc = _get_nc()
    shared = {
        "wq_t": np.ascontiguousarray(wq.T),
        "wk_t": np.ascontiguousarray(wk.T),
        "wv_t": np.ascontiguousarray(wv.T),
        "bq": np.ascontiguousarray(bq.reshape(C, 1)),
        "bk": np.ascontiguousarray(bk.reshape(C, 1)),
        "bv": np.ascontiguousarray(bv.reshape(1, C)),
    }
    in_maps = [
        {"x": np.ascontiguousarray(x[b].reshape(C, NPIX)), **shared} for b in range(B)
    ]
    res = run_bass_kernel_spmd(nc, in_maps, core_ids=list(range(B)))
    out = np.stack([res.results[b]["out"] for b in range(B)])
    return out.reshape(B, C, 64, 64)
